# revision 32
# baseline (speedup 1.0000x reference)
"""AlloLayer forward on 8 TRN2 NeuronCores.

Math: reference computes
    lp   = log_softmax(hs, -1)                      # [B,T,C]
    ex   = exp(lp[..., phone_lab] + alloW)          # [B,T,A]
    sq   = scatter_add(ex, phoneme_lab)             # [B,T,P]
    red  = (sq.sum(-1) - 1) / P
    out  = log(sq - red)

The gather+exp+scatter collapses into a matmul: with
    M[c,p] = sum_{a: phone_lab[a]==c, phoneme_lab[a]==p} exp(alloW[a])
we have sq = softmax(hs) @ M.  Augment M with column 256 = M @ 1 (row sums)
and column 257 = ones, then per 128-row block:
    U = exp(X) @ Maug          (PE, f32r)
    s = U[:,257]  w = U[:,256]
    out = Ln(U[:,0:256] * (1/s) + (s - w)/(P*s))    (one ACT op)

Data-parallel over B*T rows: 16384 rows -> 2048 rows per core, no
cross-core communication; output gathered on host.
"""

import os
import numpy as np

import concourse.bass as bass
import concourse.tile as tile
from concourse import bacc, mybir
from concourse import bass_utils

F32 = mybir.dt.float32
F32R = mybir.dt.float32r
BF16 = mybir.dt.bfloat16

N_CORES = 8
B, T, C, A, P = 16, 1024, 512, 4096, 256
ROWS = B * T                      # 16384
R_PER_CORE = ROWS // N_CORES      # 2048
NAUG = P + 2                      # 258: [M | M@1 | ones]
BLK = 128
NBLK = R_PER_CORE // BLK          # 16
SUPER = 4                         # row-blocks per DMA superblock
NSUPER = NBLK // SUPER            # 4
KCH = C // BLK                    # 4 contraction chunks


def _pin_act_table(arch):
    """Make natural_log_exp_and_others the only table-set advertising Exp/Ln.

    The compiled NEFF then keeps one resident ACT table set for the whole
    kernel instead of reloading (~1.3us each) on every Exp<->Ln alternation.
    Mutates the functools-cached dict in place; set indices are unchanged and
    the real HW set genuinely contains both functions, so execution is
    unaffected.
    """
    from concourse import hw_specs

    tabs = hw_specs.get_activation_tables(arch)
    both = "natural_log_exp_and_others"
    assert both in tabs
    af = mybir.ActivationFunctionType
    assert af.Exp in tabs[both] and af.Ln in tabs[both]
    for name, fns in tabs.items():
        if name != both:
            fns.discard(af.Exp)
            fns.discard(af.Ln)


def build_graph(x_bufs=4, e_bufs=3, o_bufs=3, xt_bufs=3, u_bufs=2, out_batch=2,
                bench_iters=0, dma_only=False, compute_only=False,
                out_on_sync=False, bf16_t=False, cast_eng="vector"):
    from contextlib import ExitStack, nullcontext

    nc = bacc.Bacc("TRN2", target_bir_lowering=False, debug=False, num_devices=1)
    _pin_act_table(nc.m.arch)
    x_ap = nc.dram_tensor("x", [R_PER_CORE, C], F32, kind="ExternalInput").ap()
    maug_ap = nc.dram_tensor("maug", [KCH, BLK, NAUG], F32, kind="ExternalInput").ap()
    ident_ap = nc.dram_tensor("ident", [BLK, BLK], F32, kind="ExternalInput").ap()
    out_ap = nc.dram_tensor("out", [R_PER_CORE, P], F32, kind="ExternalOutput").ap()

    # Pair schedule: 1-block pairs at both ends — fast pipeline fill at the
    # start, short drain chain at the end.
    PAIRS = [1, 1] + [2] * ((NBLK - 4) // 2) + [1, 1]
    assert sum(PAIRS) == NBLK

    with tile.TileContext(nc) as tc, ExitStack() as ctx:
        const_pool = ctx.enter_context(tc.tile_pool(name="const", bufs=1))
        x_pool = ctx.enter_context(tc.tile_pool(name="xin", bufs=x_bufs))
        e_pool = ctx.enter_context(tc.tile_pool(name="e", bufs=e_bufs))
        o_pool = ctx.enter_context(tc.tile_pool(name="o", bufs=o_bufs))
        s_pool = ctx.enter_context(tc.tile_pool(name="small", bufs=4))
        xt_pool = ctx.enter_context(tc.tile_pool(name="xt", bufs=xt_bufs, space="PSUM"))
        u_pool = ctx.enter_context(tc.tile_pool(name="u", bufs=u_bufs, space="PSUM"))

        # Constants go over SWDGE (Pool) so they don't delay the HWDGE x loads.
        ident_sb = const_pool.tile([BLK, BLK], F32)
        nc.gpsimd.dma_start(ident_sb[:], ident_ap[:, :])
        ident_bf = None
        xb_pool = None
        if bf16_t:
            ident_bf = const_pool.tile([BLK, BLK], BF16)
            nc.gpsimd.dma_start(ident_bf[:], ident_ap[:, :])
            xb_pool = ctx.enter_context(tc.tile_pool(name="xb", bufs=3))
        maug_sb = const_pool.tile([BLK, KCH * NAUG], BF16)
        nc.gpsimd.dma_start(
            maug_sb[:].rearrange("p (k n) -> p k n", n=NAUG),
            maug_ap[:, :, :].rearrange("k p n -> p k n"),
        )

        # bench_iters>0 wraps the whole body in an on-device loop so the
        # per-iteration time can be resolved through the ~1s axon RPC noise.
        # bench_iters < 0: staggered-reset back-edge (cross-iter overlap)
        loop_cm = (
            tc.For_i(0, abs(bench_iters), 1, staggered_reset=bench_iters < 0)
            if bench_iters
            else nullcontext()
        )
        ctx.enter_context(loop_cm)

        row0 = 0
        outs = None
        ob_blocks = 0       # blocks accumulated in current outs tile
        ob_row0 = 0         # first row-block covered by current outs tile
        OBW = out_batch * 2  # max blocks per outs tile

        def flush_outs():
            nonlocal outs, ob_blocks, ob_row0
            if outs is None or ob_blocks == 0:
                return
            # output DMA via SWDGE (Pool) — separate queue from the inputs
            if compute_only:
                outs = None
                ob_blocks = 0
                return
            out_eng = nc.sync if out_on_sync else nc.gpsimd
            out_eng.dma_start(
                out_ap[ob_row0 * BLK:(ob_row0 + ob_blocks) * BLK, :].rearrange(
                    "(b p) c -> p b c", p=BLK
                ),
                outs[:, 0:ob_blocks * P].rearrange("p (b c) -> p b c", c=P),
            )
            outs = None
            ob_blocks = 0

        for pn in PAIRS:
            # input DMA for this pair on the SP HWDGE ring (in-DMAs only, so
            # no out-DMA can head-of-line block the input stream)
            xs = x_pool.tile([BLK, 2 * C], F32, tag="xs")
            if compute_only:
                nc.vector.memset(xs[:, 0:8], 0.0)  # mark tile written
            else:
                nc.sync.dma_start(
                    xs[:, 0:pn * C].rearrange("p (b c) -> p b c", c=C),
                    x_ap[row0 * BLK:(row0 + pn) * BLK, :].rearrange(
                        "(b p) c -> p b c", p=BLK
                    ),
                )
            if outs is None:
                outs = o_pool.tile([BLK, OBW * P], F32, tag="outs")
                ob_row0 = row0
            if dma_only:
                if ob_blocks == 0:
                    nc.vector.memset(outs[:, 0:8], 0.0)  # mark tile written
                ob_blocks += pn
                row0 += pn
                if ob_blocks + 2 > OBW:
                    flush_outs()
                continue
            if bf16_t:
                # cast x to bf16 on an otherwise-idle engine; PE transposes
                # then run at 1 cycle/row (vs 2 for f32) and xt PSUM tiles
                # take 1 bank instead of 2
                xb = xb_pool.tile([BLK, 2 * C], BF16, tag="xb")
                getattr(nc, cast_eng).tensor_copy(xb[:, 0:pn * C], xs[:, 0:pn * C])
                t_src, t_ident, t_dt = xb, ident_bf, BF16
            else:
                t_src, t_ident, t_dt = xs, ident_sb, F32
            xt = xt_pool.tile([BLK, 2 * C], t_dt, tag="xt")  # PSUM
            for bb in range(pn):
                for k in range(KCH):
                    nc.tensor.transpose(
                        xt[:, bb * C + k * BLK:bb * C + (k + 1) * BLK],
                        t_src[:, bb * C + k * BLK:bb * C + (k + 1) * BLK],
                        t_ident[:],
                    )
            e = e_pool.tile([BLK, 2 * C], BF16, tag="e")  # exp(x), [c, r] layout
            nc.scalar.activation(
                e[:, 0:pn * C], xt[:, 0:pn * C],
                mybir.ActivationFunctionType.Exp,
            )
            for bb in range(pn):
                u = u_pool.tile([BLK, NAUG], F32, tag="u")
                for k in range(KCH):
                    nc.tensor.matmul(
                        u[:],
                        e[:, bb * C + k * BLK:bb * C + (k + 1) * BLK],
                        maug_sb[:, k * NAUG:(k + 1) * NAUG],
                        start=(k == 0),
                        stop=(k == KCH - 1),
                    )
                inv_s = s_pool.tile([BLK, 1], F32, tag="inv")
                nc.vector.reciprocal(inv_s[:], u[:, NAUG - 1:NAUG])
                m_t = s_pool.tile([BLK, 1], F32, tag="m")
                nc.vector.tensor_scalar_mul(m_t[:], inv_s[:], -1.0 / P)
                bias_t = s_pool.tile([BLK, 1], F32, tag="bias")
                # bias = (w - s) * (-inv_s/P) = (s - w)/(P*s)
                nc.vector.scalar_tensor_tensor(
                    bias_t[:],
                    u[:, P:P + 1],
                    u[:, NAUG - 1:NAUG],
                    m_t[:],
                    op0=mybir.AluOpType.subtract,
                    op1=mybir.AluOpType.mult,
                )
                ob = ob_blocks + bb
                nc.scalar.activation(
                    outs[:, ob * P:(ob + 1) * P],
                    u[:, 0:P],
                    mybir.ActivationFunctionType.Ln,
                    bias=bias_t[:],
                    scale=inv_s[:],
                )
            ob_blocks += pn
            row0 += pn
            if ob_blocks + 2 > OBW:
                flush_outs()
        flush_outs()
    nc.compile()
    return nc


def build_graph_t(x_bufs=3, e_bufs=3, o_bufs=3, u_bufs=6, out_batch=1,
                  rs=512, in_split=2, exp_split=1, out_eng="sync",
                  rsched=None, bench_iters=0, skip_mm=False, skip_dve=False):
    """Variant taking the per-core x shard PRE-TRANSPOSED on the host:
    x_t[C, R_PER_CORE].  No on-chip transposes: DMA loads [128c, r] tiles
    directly, exp runs on big tiles, PE does only the matmuls.
    """
    from contextlib import ExitStack, nullcontext

    nc = bacc.Bacc("TRN2", target_bir_lowering=False, debug=False, num_devices=1)
    _pin_act_table(nc.m.arch)
    x_ap = nc.dram_tensor("x", [C, R_PER_CORE], F32, kind="ExternalInput").ap()
    maug_ap = nc.dram_tensor("maug", [KCH, BLK, NAUG], F32, kind="ExternalInput").ap()
    out_ap = nc.dram_tensor("out", [R_PER_CORE, P], F32, kind="ExternalOutput").ap()

    if rsched is None:
        rsched = [rs] * (R_PER_CORE // rs)
    assert sum(rsched) == R_PER_CORE

    with tile.TileContext(nc) as tc, ExitStack() as ctx:
        const_pool = ctx.enter_context(tc.tile_pool(name="const", bufs=1))
        x_pool = ctx.enter_context(tc.tile_pool(name="xin", bufs=x_bufs))
        e_pool = ctx.enter_context(tc.tile_pool(name="e", bufs=e_bufs))
        o_pool = ctx.enter_context(tc.tile_pool(name="o", bufs=o_bufs))
        s_pool = ctx.enter_context(tc.tile_pool(name="small", bufs=4))
        u_pool = ctx.enter_context(tc.tile_pool(name="u", bufs=u_bufs, space="PSUM"))

        maug_sb = const_pool.tile([BLK, KCH * NAUG], BF16)
        nc.gpsimd.dma_start(
            maug_sb[:].rearrange("p (k n) -> p k n", n=NAUG),
            maug_ap[:, :, :].rearrange("k p n -> p k n"),
        )

        loop_cm = (
            tc.For_i(0, abs(bench_iters), 1) if bench_iters else nullcontext()
        )
        ctx.enter_context(loop_cm)

        x_t3 = x_ap.rearrange("(k p) r -> k p r", p=BLK)   # [KCH, 128, R]
        kper = KCH // in_split                              # c-chunks per in-DMA
        RSMAX = max(rsched)
        r0 = 0
        for rs in rsched:
            BPRS = rs // BLK
            # x slice [128, KCH*rs]: c-chunk k occupies cols [k*rs, (k+1)*rs)
            xs = x_pool.tile([BLK, KCH * RSMAX], F32, tag="xs")
            for d in range(in_split):
                nc.sync.dma_start(
                    xs[:, d * kper * rs:(d + 1) * kper * rs].rearrange(
                        "p (k r) -> p k r", r=rs
                    ),
                    x_t3[d * kper:(d + 1) * kper, :, r0:r0 + rs].rearrange(
                        "k p r -> p k r"
                    ),
                )
            e = e_pool.tile([BLK, KCH * RSMAX], BF16, tag="e")
            estep = KCH * rs // exp_split
            for s in range(exp_split):
                nc.scalar.activation(
                    e[:, s * estep:(s + 1) * estep],
                    xs[:, s * estep:(s + 1) * estep],
                    mybir.ActivationFunctionType.Exp,
                )
            outs = o_pool.tile([BLK, (RSMAX // BLK) * P], F32, tag="outs")
            if skip_mm:
                nc.vector.memset(outs[:, 0:8], 0.0)
            for b in range(BPRS if not skip_mm else 0):
                u = u_pool.tile([BLK, NAUG], F32, tag="u")
                for k in range(KCH):
                    nc.tensor.matmul(
                        u[:],
                        e[:, k * rs + b * BLK:k * rs + (b + 1) * BLK],
                        maug_sb[:, k * NAUG:(k + 1) * NAUG],
                        start=(k == 0),
                        stop=(k == KCH - 1),
                    )
                if skip_dve:
                    nc.scalar.activation(
                        outs[:, b * P:(b + 1) * P],
                        u[:, 0:P],
                        mybir.ActivationFunctionType.Ln,
                        bias=0.0,
                        scale=1.0,
                    )
                    continue
                inv_s = s_pool.tile([BLK, 1], F32, tag="inv")
                nc.vector.reciprocal(inv_s[:], u[:, NAUG - 1:NAUG])
                m_t = s_pool.tile([BLK, 1], F32, tag="m")
                nc.vector.tensor_scalar_mul(m_t[:], inv_s[:], -1.0 / P)
                bias_t = s_pool.tile([BLK, 1], F32, tag="bias")
                nc.vector.scalar_tensor_tensor(
                    bias_t[:],
                    u[:, P:P + 1],
                    u[:, NAUG - 1:NAUG],
                    m_t[:],
                    op0=mybir.AluOpType.subtract,
                    op1=mybir.AluOpType.mult,
                )
                nc.scalar.activation(
                    outs[:, b * P:(b + 1) * P],
                    u[:, 0:P],
                    mybir.ActivationFunctionType.Ln,
                    bias=bias_t[:],
                    scale=inv_s[:],
                )
            getattr(nc, out_eng).dma_start(
                out_ap[r0:r0 + rs, :].rearrange("(b p) c -> p b c", p=BLK),
                outs[:, 0:BPRS * P].rearrange("p (b c) -> p b c", c=P),
            )
            r0 += rs
    nc.compile()
    return nc


def make_maug(alloW, phone_arc_labels, phoneme_arc_labels):
    alloW = np.asarray(alloW, dtype=np.float64).reshape(-1)
    phone = np.asarray(phone_arc_labels).astype(np.int64).reshape(-1)
    phoneme = np.asarray(phoneme_arc_labels).astype(np.int64).reshape(-1)
    M = np.zeros((C, P), dtype=np.float64)
    np.add.at(M, (phone, phoneme), np.exp(alloW))
    maug = np.empty((C, NAUG), dtype=np.float64)
    maug[:, :P] = M
    maug[:, P] = M.sum(axis=1)
    maug[:, P + 1] = 1.0
    return maug.astype(np.float32).reshape(KCH, BLK, NAUG)


_NC = None


def _get_nc():
    global _NC
    if _NC is None:
        _NC = build_graph()
    return _NC


def run(hs_pad, alloW, phone_arc_labels, phoneme_arc_labels, n_phonemes, trace=False):
    assert int(n_phonemes) == P
    hs = np.ascontiguousarray(np.asarray(hs_pad, dtype=np.float32)).reshape(ROWS, C)
    maug = make_maug(alloW, phone_arc_labels, phoneme_arc_labels)
    ident = np.eye(BLK, dtype=np.float32)
    in_maps = [
        {
            "x": hs[i * R_PER_CORE:(i + 1) * R_PER_CORE],
            "maug": maug,
            "ident": ident,
        }
        for i in range(N_CORES)
    ]
    res = bass_utils.run_bass_kernel_spmd(
        _get_nc(), in_maps, core_ids=list(range(N_CORES)), trace=trace
    )
    out = np.concatenate([res.results[i]["out"] for i in range(N_CORES)], axis=0)
    return out.reshape(B, T, P), res


def kernel(hs_pad, alloW, phone_arc_labels, phoneme_arc_labels, n_phonemes):
    out, _ = run(hs_pad, alloW, phone_arc_labels, phoneme_arc_labels, n_phonemes)
    return out
